# revision 4
# baseline (speedup 1.0000x reference)
"""LSTMCell-variant Bass kernel for 8 Trainium2 NeuronCores.

Reference computation (B = H = O = 2048, fp32):
    z_g  = h @ W_hg + x @ W_xg + b_xg          (4 gates g in {f,g,i,o})
    gate = act(LayerNorm(z_g))                  (sigmoid/tanh/sigmoid/sigmoid)
    c_t  = f @ c_states + g @ i                 (matmul gating, not elementwise)
    h_t  = tanh(c_t) @ o
    y_t  = h_t @ W_y + b_y
    returns (c_t, h_t, y_t)

Strategy: data-parallel over batch rows (256 rows/core). Each core computes
its 4 gate row-shards; i and o are AllGathered (they are the RIGHT operands
of the gate matmuls, so every core needs them in full); f, g, tanh(c_t), h_t
are only needed as row shards (transposed locally on the PE for use as the
stationary matmul operand).

All matmul operands are bf16 (fp32 PSUM accumulate): halves the dominant
HBM stream (the 8 gate projection weights) vs fp32r and enables the PE's
fast-weight-load path. Scheduling keeps the PE dense (any >3.4us idle
window halves the PE clock via the HAM activity monitor):
  - gate order i, o, f, g so both AllGathers overlap the f/g projections;
  - stage 2 runs all f@c chunks before g@i so the i-gather has extra slack;
  - the gathered o is prefetched into SBUF during stage 2, making stage 3
    DMA-free so the W_y stream for stage 4 owns the bandwidth;
  - LayerNorm uses the E[z^2]-m^2 form: both reductions read the raw z in
    parallel (Vector + Scalar) and the normalize+activate folds into one
    Scalar activation via its per-partition scale/bias operands.

Host-side staging (free w.r.t. HW time): x and h are fed pre-transposed
(contraction dim on partitions); per-column bias vectors are fed
pre-replicated to 128 partitions; all dtype casts happen on the host.
"""

import os
from contextlib import ExitStack

import numpy as np

os.environ.setdefault("MYCRO_LOCAL_CACHE", "1")

try:
    import concourse.bass as bass  # noqa: F401
except ImportError:  # pragma: no cover
    import sys

    sys.path.insert(0, "/opt/trn_rl_repo")
    import concourse.bass as bass  # noqa: F401

import concourse.mybir as mybir
import concourse.tile as tile
from concourse import bacc
from concourse.bass_utils import run_bass_kernel_spmd
from concourse.masks import make_identity

B = 2048
H = 2048
OD = 2048
NCORES = 8
BS = B // NCORES  # 256 batch rows per core
NB = BS // 128  # 2 row-chunks of 128
KT = H // 128  # 16 contraction chunks
NSL = 4  # moving slices of 512 per full-width strip
EPS = 1e-5

F32 = mybir.dt.float32
BF16 = mybir.dt.bfloat16
AX = mybir.AxisListType
OP = mybir.AluOpType
AF = mybir.ActivationFunctionType

_cache = {}


def _body(ctx: ExitStack, tc, I, Outs, apply_affine: bool):
    nc = tc.nc

    const = ctx.enter_context(tc.tile_pool(name="const", bufs=1))
    persist = ctx.enter_context(tc.tile_pool(name="persist", bufs=1))
    wmov = ctx.enter_context(tc.tile_pool(name="wmov", bufs=6))
    bxp = ctx.enter_context(tc.tile_pool(name="bxp", bufs=2))
    rows = ctx.enter_context(tc.tile_pool(name="rows", bufs=2))
    stats = ctx.enter_context(tc.tile_pool(name="stats", bufs=6))
    zps = ctx.enter_context(tc.tile_pool(name="zps", bufs=8, space="PSUM"))
    dram = ctx.enter_context(tc.tile_pool(name="dram", bufs=1, space="DRAM"))

    ident = const.tile([128, 128], F32, tag="ident", name="ident")
    make_identity(nc, ident[:])
    ident_b = const.tile([128, 128], BF16, tag="ident_b", name="ident_b")
    nc.vector.tensor_copy(ident_b[:], ident[:])
    epsb = const.tile([128, 1], F32, tag="epsb", name="epsb")
    nc.gpsimd.memset(epsb[:], EPS)

    # Persistent k-major activations: [128 partitions, KT chunks, col block].
    # Column block kc of hT holds hT[kc*128:(kc+1)*128, :] i.e. contraction
    # rows on partitions, ready to slice as a [128, 128] stationary operand.
    def kmajor(name, tag=None, cols=BS):
        return persist.tile([128, KT, cols], BF16, tag=tag or name, name=name)

    hT = kmajor("hT")
    xT = kmajor("xT")
    fT = kmajor("fT")
    gT = kmajor("gT")
    # hT/xT are dead once the gate projections finish; tcT/htT are only
    # written afterwards, so they share the same SBUF slots.
    tcT = kmajor("tcT", tag="hT")
    htT = kmajor("htT", tag="xT")
    # Full gathered o gate, prefetched to SBUF during stage 2 so stage 3
    # issues no DMA at all (the W_y stream for stage 4 gets the bandwidth).
    oP = kmajor("oP", cols=H)

    # DRAM bounce buffers for the i/o AllGathers.
    io_in = {g: dram.tile([BS, H], BF16, tag=f"io_in_{g}", name=f"io_in_{g}") for g in "io"}
    io_full = {g: dram.tile([B, H], BF16, tag=f"io_full_{g}", name=f"io_full_{g}", addr_space="Shared") for g in "io"}

    def layernorm_act(z_sb, func, gate_out, ga_sb, be_sb):
        """z_sb [128, H] -> gate_out [128, H] = func(LN(z)) (affine optional).

        Uses var = E[z^2] - mean^2 so the two reductions over z run
        concurrently on Vector and Scalar, then normalize+activate is a
        single Scalar pass: func(z*inv + (-m*inv)).
        """
        s1 = stats.tile([128, 1], F32, tag="s1", name="s1")
        nc.vector.tensor_reduce(s1[:], z_sb[:], AX.X, OP.add)
        trash = rows.tile([128, H], BF16, tag="gate", name="trash")
        ssq = stats.tile([128, 1], F32, tag="ssq", name="ssq")
        nc.scalar.activation(trash[:], z_sb[:], AF.Square, accum_out=ssq[:])
        m = stats.tile([128, 1], F32, tag="m", name="m")
        nc.vector.tensor_scalar_mul(m[:], s1[:], 1.0 / H)
        v = stats.tile([128, 1], F32, tag="v", name="v")
        nc.vector.tensor_scalar_mul(v[:], ssq[:], 1.0 / H)
        mm = stats.tile([128, 1], F32, tag="mm", name="mm")
        nc.vector.tensor_tensor(mm[:], m[:], m[:], OP.mult)
        nc.vector.tensor_tensor(v[:], v[:], mm[:], OP.subtract)
        std = stats.tile([128, 1], F32, tag="std", name="std")
        nc.scalar.activation(std[:], v[:], AF.Sqrt, bias=epsb[:])
        inv = stats.tile([128, 1], F32, tag="inv", name="inv")
        nc.vector.reciprocal(inv[:], std[:])
        nmi = stats.tile([128, 1], F32, tag="nmi", name="nmi")
        nc.vector.tensor_tensor(nmi[:], m[:], inv[:], OP.mult)
        nc.vector.tensor_scalar_mul(nmi[:], nmi[:], -1.0)
        if not apply_affine:
            nc.scalar.activation(gate_out[:], z_sb[:], func, scale=inv[:], bias=nmi[:])
        else:
            nc.vector.tensor_scalar(
                out=z_sb[:], in0=z_sb[:], scalar1=inv[:], scalar2=nmi[:],
                op0=OP.mult, op1=OP.add,
            )
            nc.vector.tensor_tensor(z_sb[:], z_sb[:], ga_sb[:], OP.mult)
            nc.vector.tensor_tensor(z_sb[:], z_sb[:], be_sb[:], OP.add)
            nc.scalar.activation(gate_out[:], z_sb[:], func)

    def transpose_rows(src_sb, dstT, b):
        """src_sb [128, H] (row-chunk b) -> dstT[:, :, b*128:(b+1)*128]."""
        for kc in range(KT):
            tp = zps.tile([128, 128], BF16, tag="z", name="tp")
            nc.tensor.transpose(tp[:], src_sb[:, kc * 128 : (kc + 1) * 128], ident_b[:])
            nc.vector.tensor_copy(dstT[:, kc, b * 128 : (b + 1) * 128], tp[:])

    def accumulate(psums, stat_list, strips, start=True, stop=True, sbuf_mov=None,
                   pre_dma=None):
        """Accumulate sum_k of lhsT.T @ rhs into psums[b][j].

        strips: DRAM sources streamed as full-width [128, H] tiles (0.5 MB
        contiguous DMAs keep the HW DGE at large-packet throughput), or
        sbuf_mov: a resident k-major [128, KT, H] SBUF tile used directly.
        Loop order keeps each stationary tile resident for NSL consecutive
        moving slices. pre_dma(kc) lets the caller interleave extra loads.
        """
        NP = len(stat_list)
        for kc in range(KT):
            if pre_dma is not None:
                pre_dma(kc)
            mov = []
            if sbuf_mov is not None:
                mov = [sbuf_mov[:, kc, :]]
            else:
                for dram_src in strips:
                    w = wmov.tile([128, H], BF16, tag="wm", name="wm")
                    nc.sync.dma_start(w[:], dram_src[kc * 128 : (kc + 1) * 128, :])
                    mov.append(w[:])
            for p, statT in enumerate(stat_list):
                for b in range(NB):
                    for j in range(NSL):
                        nc.tensor.matmul(
                            psums[b][j][:],
                            statT[:, kc, b * 128 : (b + 1) * 128],
                            mov[p][:, j * 512 : (j + 1) * 512],
                            start=(start and kc == 0 and p == 0),
                            stop=(stop and kc == KT - 1 and p == NP - 1),
                        )

    # ---- Stage 1: the four gates (i, o first so their AllGathers overlap f,g)
    gate_specs = [
        ("i", AF.Sigmoid),
        ("o", AF.Sigmoid),
        ("f", AF.Sigmoid),
        ("g", AF.Tanh),
    ]
    for gi, (gname, func) in enumerate(gate_specs):
        bx_sb = bxp.tile([128, H], BF16, tag="bx", name="bx")
        nc.sync.dma_start(bx_sb[:], I[f"bx_{gname}"][:])
        ga_sb = be_sb = None
        if apply_affine:
            ga_sb = bxp.tile([128, H], BF16, tag="ga", name="ga")
            nc.sync.dma_start(ga_sb[:], I[f"ga_{gname}"][:])
            be_sb = bxp.tile([128, H], BF16, tag="be", name="be")
            nc.sync.dma_start(be_sb[:], I[f"be_{gname}"][:])

        z_sb = [rows.tile([128, H], F32, tag="z_sb", name="z_sb") for _ in range(NB)]
        psums = [
            [zps.tile([128, 512], F32, tag="z", name="z") for _ in range(NSL)]
            for _ in range(NB)
        ]

        # Interleave the persistent hT/xT chunk loads into the first gate's
        # streaming loop so the PE's first matmul only waits for 4 small DMAs.
        pre = None
        if gi == 0:
            def pre(kc):
                nc.sync.dma_start(hT[:, kc, :], I["hT"][kc * 128 : (kc + 1) * 128, :])
                nc.sync.dma_start(xT[:, kc, :], I["xT"][kc * 128 : (kc + 1) * 128, :])

        accumulate(
            psums,
            [hT, xT],
            [I[f"W_h{gname}"], I[f"W_x{gname}"]],
            pre_dma=pre,
        )
        for b in range(NB):
            for j in range(NSL):
                col = slice(j * 512, (j + 1) * 512)
                nc.vector.tensor_tensor(
                    z_sb[b][:, col], psums[b][j][:], bx_sb[:, col], OP.add
                )
        for b in range(NB):
            gt = rows.tile([128, H], BF16, tag="gate", name="gate")
            layernorm_act(z_sb[b], func, gt, ga_sb, be_sb)
            if gname in ("i", "o"):
                nc.sync.dma_start(io_in[gname][b * 128 : (b + 1) * 128, :], gt[:])
            else:
                transpose_rows(gt, fT if gname == "f" else gT, b)
        if gname in ("i", "o"):
            nc.gpsimd.collective_compute(
                "AllGather",
                OP.bypass,
                replica_groups=[list(range(NCORES))],
                ins=[io_in[gname].opt()],
                outs=[io_full[gname].opt()],
            )

    # ---- Stage 2: c_t = f @ c_states + g @ i ; tanh + transpose
    # Split: all f@c chunks first (no collective dependency), then g@i.
    c_sb = [rows.tile([128, H], BF16, tag="c_sb", name="c_sb") for _ in range(NB)]
    tc_sb = [rows.tile([128, H], BF16, tag="z_sb", name="tc_sb") for _ in range(NB)]
    psums = [
        [zps.tile([128, 512], F32, tag="z", name="z") for _ in range(NSL)]
        for _ in range(NB)
    ]
    accumulate(psums, [fT], [I["c_states"]], start=True, stop=False)
    accumulate(psums, [gT], [io_full["i"]], start=False, stop=True)
    # Prefetch the gathered o into SBUF; these sit on the Sync queue after
    # stage 2's strip loads, so they run during stage-2 compute.
    for kc in range(KT):
        nc.sync.dma_start(oP[:, kc, :], io_full["o"][kc * 128 : (kc + 1) * 128, :])
    for b in range(NB):
        for j in range(NSL):
            col = slice(j * 512, (j + 1) * 512)
            nc.vector.tensor_copy(c_sb[b][:, col], psums[b][j][:])
            nc.scalar.activation(tc_sb[b][:, col], psums[b][j][:], AF.Tanh)
    for b in range(NB):
        nc.sync.dma_start(Outs["c_out"][b * 128 : (b + 1) * 128, :], c_sb[b][:])
        transpose_rows(tc_sb[b], tcT, b)

    # ---- Stage 3: h_t = tanh(c_t) @ o ; transpose (o is SBUF-resident)
    h_sb = [rows.tile([128, H], BF16, tag="gate", name="h_sb") for _ in range(NB)]
    psums = [
        [zps.tile([128, 512], F32, tag="z", name="z") for _ in range(NSL)]
        for _ in range(NB)
    ]
    accumulate(psums, [tcT], None, sbuf_mov=oP)
    for b in range(NB):
        for j in range(NSL):
            col = slice(j * 512, (j + 1) * 512)
            nc.vector.tensor_copy(h_sb[b][:, col], psums[b][j][:])
    for b in range(NB):
        nc.sync.dma_start(Outs["h_out"][b * 128 : (b + 1) * 128, :], h_sb[b][:])
        transpose_rows(h_sb[b], htT, b)

    # ---- Stage 4: y = h_t @ W_y + b_y  (W_y streams during stage 3)
    by_sb = bxp.tile([128, OD], BF16, tag="bx", name="bx")
    nc.sync.dma_start(by_sb[:], I["by_rep"][:])
    y_sb = [rows.tile([128, OD], BF16, tag="z_sb", name="y_sb") for _ in range(NB)]
    psums = [
        [zps.tile([128, 512], F32, tag="z", name="z") for _ in range(NSL)]
        for _ in range(NB)
    ]
    accumulate(psums, [htT], [I["W_y"]])
    for b in range(NB):
        for j in range(NSL):
            col = slice(j * 512, (j + 1) * 512)
            nc.vector.tensor_tensor(
                y_sb[b][:, col], psums[b][j][:], by_sb[:, col], OP.add
            )
    for b in range(NB):
        nc.sync.dma_start(Outs["y_out"][b * 128 : (b + 1) * 128, :], y_sb[b][:])


def _build(apply_affine: bool):
    nc = bacc.Bacc(
        "TRN2",
        target_bir_lowering=False,
        debug=False,
        enable_asserts=False,
        num_devices=NCORES,
    )
    I = {}

    def di(name, shape, dt=BF16):
        I[name] = nc.dram_tensor(name, list(shape), dt, kind="ExternalInput").ap()

    di("hT", (H, BS))
    di("xT", (H, BS))
    di("c_states", (B, H))
    di("W_y", (H, OD))
    di("by_rep", (128, OD))
    for g in "fgio":
        di(f"W_h{g}", (H, H))
        di(f"W_x{g}", (H, H))
        di(f"bx_{g}", (128, H))
        if apply_affine:
            di(f"ga_{g}", (128, H))
            di(f"be_{g}", (128, H))
    Outs = {
        n: nc.dram_tensor(n, [BS, H], BF16, kind="ExternalOutput").ap()
        for n in ("c_out", "h_out", "y_out")
    }

    with tile.TileContext(nc) as tc, ExitStack() as ctx:
        _body(ctx, tc, I, Outs, apply_affine)
    nc.compile()
    return nc


def kernel(**inputs):
    inputs = {k: np.asarray(v, dtype=np.float32) for k, v in inputs.items()}
    apply_affine = not all(
        np.all(inputs[f"g_{g}"] == 1.0) and np.all(inputs[f"be_{g}"] == 0.0)
        for g in "fgio"
    )
    if apply_affine not in _cache:
        _cache[apply_affine] = _build(apply_affine)
    nc = _cache[apply_affine]

    import ml_dtypes

    bf16 = ml_dtypes.bfloat16
    hT_full = np.ascontiguousarray(inputs["h_states"].T.astype(bf16))
    xT_full = np.ascontiguousarray(inputs["inputs"].T.astype(bf16))

    def rep(v):
        return np.ascontiguousarray(
            np.broadcast_to(v[None, :].astype(bf16), (128, v.shape[0]))
        )

    base = {
        "c_states": inputs["c_states"].astype(bf16),
        "W_y": inputs["W_y"].astype(bf16),
        "by_rep": rep(inputs["b_y"]),
    }
    for g in "fgio":
        base[f"W_h{g}"] = inputs[f"W_h{g}"].astype(bf16)
        base[f"W_x{g}"] = inputs[f"W_x{g}"].astype(bf16)
        base[f"bx_{g}"] = rep(inputs[f"b_x{g}"])
        if apply_affine:
            base[f"ga_{g}"] = rep(inputs[f"g_{g}"])
            base[f"be_{g}"] = rep(inputs[f"be_{g}"])

    in_maps = [
        dict(
            base,
            hT=np.ascontiguousarray(hT_full[:, c * BS : (c + 1) * BS]),
            xT=np.ascontiguousarray(xT_full[:, c * BS : (c + 1) * BS]),
        )
        for c in range(NCORES)
    ]

    res = run_bass_kernel_spmd(
        nc,
        in_maps,
        list(range(NCORES)),
        trace=bool(os.environ.get("KERNEL_TRACE")),
    )
    kernel.last_result = res

    def cat(name):
        return np.concatenate(
            [res.results[c][name].astype(np.float32) for c in range(NCORES)], axis=0
        )

    return (cat("c_out"), cat("h_out"), cat("y_out"))


# revision 6
# speedup vs baseline: 1.3360x; 1.3360x over previous
"""LSTMCell-variant Bass kernel for 8 Trainium2 NeuronCores.

Reference computation (B = H = O = 2048, fp32):
    z_g  = h @ W_hg + x @ W_xg + b_xg          (4 gates g in {f,g,i,o})
    gate = act(LayerNorm(z_g))                  (sigmoid/tanh/sigmoid/sigmoid)
    c_t  = f @ c_states + g @ i                 (matmul gating, not elementwise)
    h_t  = tanh(c_t) @ o
    y_t  = h_t @ W_y + b_y
    returns (c_t, h_t, y_t)

Strategy: data-parallel over batch rows (256 rows/core). Each core computes
its 4 gate row-shards; i and o are AllGathered (they are the RIGHT operands
of the gate matmuls, so every core needs them in full); f, g, tanh(c_t), h_t
are only needed as row shards (transposed locally on the PE for use as the
stationary matmul operand).

All matmul operands are bf16 (fp32 PSUM accumulate): halves the dominant
HBM stream (the 8 gate projection weights) vs fp32r and enables the PE's
fast-weight-load path. Scheduling keeps the PE dense (any >3.4us idle
window halves the PE clock via the HAM activity monitor):
  - gate order i, o, f, g so both AllGathers overlap the f/g projections;
  - stage 2 runs all f@c chunks before g@i so the i-gather has extra slack;
  - the gathered o is prefetched into SBUF during stage 2, making stage 3
    DMA-free so the W_y stream for stage 4 owns the bandwidth;
  - LayerNorm uses the E[z^2]-m^2 form: both reductions read the raw z in
    parallel (Vector + Scalar) and the normalize+activate folds into one
    Scalar activation via its per-partition scale/bias operands.

Host-side staging (free w.r.t. HW time): x and h are fed pre-transposed
(contraction dim on partitions); per-column bias vectors are fed
pre-replicated to 128 partitions; all dtype casts happen on the host.
"""

import os
from contextlib import ExitStack

import numpy as np

os.environ.setdefault("MYCRO_LOCAL_CACHE", "1")

try:
    import concourse.bass as bass  # noqa: F401
except ImportError:  # pragma: no cover
    import sys

    sys.path.insert(0, "/opt/trn_rl_repo")
    import concourse.bass as bass  # noqa: F401

import concourse.mybir as mybir
import concourse.tile as tile
from concourse import bacc
from concourse.bass_utils import run_bass_kernel_spmd
from concourse.masks import make_identity

B = 2048
H = 2048
OD = 2048
NCORES = 8
BS = B // NCORES  # 256 batch rows per core
NB = BS // 128  # 2 row-chunks of 128
KT = H // 128  # 16 contraction chunks
NSL = 4  # moving slices of 512 per full-width strip
EPS = 1e-5

F32 = mybir.dt.float32
BF16 = mybir.dt.bfloat16
AX = mybir.AxisListType
OP = mybir.AluOpType
AF = mybir.ActivationFunctionType

_cache = {}


def _body(ctx: ExitStack, tc, I, Outs, apply_affine: bool):
    nc = tc.nc

    const = ctx.enter_context(tc.tile_pool(name="const", bufs=1))
    persist = ctx.enter_context(tc.tile_pool(name="persist", bufs=1))
    wmov = ctx.enter_context(tc.tile_pool(name="wmov", bufs=6))
    bxp = ctx.enter_context(tc.tile_pool(name="bxp", bufs=2))
    rows = ctx.enter_context(tc.tile_pool(name="rows", bufs=2))
    stats = ctx.enter_context(tc.tile_pool(name="stats", bufs=6))
    zps = ctx.enter_context(tc.tile_pool(name="zps", bufs=8, space="PSUM"))
    dram = ctx.enter_context(tc.tile_pool(name="dram", bufs=1, space="DRAM"))

    ident = const.tile([128, 128], F32, tag="ident", name="ident")
    make_identity(nc, ident[:])
    ident_b = const.tile([128, 128], BF16, tag="ident_b", name="ident_b")
    nc.vector.tensor_copy(ident_b[:], ident[:])
    epsb = const.tile([128, 1], F32, tag="epsb", name="epsb")
    nc.gpsimd.memset(epsb[:], EPS)

    # Persistent k-major activations: [128 partitions, KT chunks, col block].
    # Column block kc of hT holds hT[kc*128:(kc+1)*128, :] i.e. contraction
    # rows on partitions, ready to slice as a [128, 128] stationary operand.
    def kmajor(name, tag=None, cols=BS):
        return persist.tile([128, KT, cols], BF16, tag=tag or name, name=name)

    hT = kmajor("hT")
    xT = kmajor("xT")
    fT = kmajor("fT")
    gT = kmajor("gT")
    # hT/xT are dead once the gate projections finish; tcT/htT are only
    # written afterwards, so they share the same SBUF slots.
    tcT = kmajor("tcT", tag="hT")
    htT = kmajor("htT", tag="xT")
    # Full gathered o gate, prefetched to SBUF during stage 2 so stage 3
    # issues no DMA at all (the W_y stream for stage 4 gets the bandwidth).
    oP = kmajor("oP", cols=H)

    # DRAM bounce buffers for the i/o AllGathers.
    io_in = {g: dram.tile([BS, H], BF16, tag=f"io_in_{g}", name=f"io_in_{g}") for g in "io"}
    io_full = {g: dram.tile([B, H], BF16, tag=f"io_full_{g}", name=f"io_full_{g}", addr_space="Shared") for g in "io"}

    def layernorm_act(z_sb, func, gate_out, ga_sb, be_sb):
        """z_sb [128, H] -> gate_out [128, H] = func(LN(z)) (affine optional).

        Uses var = E[z^2] - mean^2 so the two reductions over z run
        concurrently on Vector and Scalar, then normalize+activate is a
        single Scalar pass: func(z*inv + (-m*inv)).
        """
        s1 = stats.tile([128, 1], F32, tag="s1", name="s1")
        nc.vector.tensor_reduce(s1[:], z_sb[:], AX.X, OP.add)
        trash = rows.tile([128, H], BF16, tag="gate", name="trash")
        ssq = stats.tile([128, 1], F32, tag="ssq", name="ssq")
        nc.scalar.activation(trash[:], z_sb[:], AF.Square, accum_out=ssq[:])
        m = stats.tile([128, 1], F32, tag="m", name="m")
        nc.vector.tensor_scalar_mul(m[:], s1[:], 1.0 / H)
        v = stats.tile([128, 1], F32, tag="v", name="v")
        nc.vector.tensor_scalar_mul(v[:], ssq[:], 1.0 / H)
        mm = stats.tile([128, 1], F32, tag="mm", name="mm")
        nc.vector.tensor_tensor(mm[:], m[:], m[:], OP.mult)
        nc.vector.tensor_tensor(v[:], v[:], mm[:], OP.subtract)
        std = stats.tile([128, 1], F32, tag="std", name="std")
        nc.scalar.activation(std[:], v[:], AF.Sqrt, bias=epsb[:])
        inv = stats.tile([128, 1], F32, tag="inv", name="inv")
        nc.vector.reciprocal(inv[:], std[:])
        nmi = stats.tile([128, 1], F32, tag="nmi", name="nmi")
        nc.vector.tensor_tensor(nmi[:], m[:], inv[:], OP.mult)
        nc.vector.tensor_scalar_mul(nmi[:], nmi[:], -1.0)
        if not apply_affine:
            nc.scalar.activation(gate_out[:], z_sb[:], func, scale=inv[:], bias=nmi[:])
        else:
            nc.vector.tensor_scalar(
                out=z_sb[:], in0=z_sb[:], scalar1=inv[:], scalar2=nmi[:],
                op0=OP.mult, op1=OP.add,
            )
            nc.vector.tensor_tensor(z_sb[:], z_sb[:], ga_sb[:], OP.mult)
            nc.vector.tensor_tensor(z_sb[:], z_sb[:], be_sb[:], OP.add)
            nc.scalar.activation(gate_out[:], z_sb[:], func)

    def transpose_rows(src_sb, dstT, b):
        """src_sb [128, H] (row-chunk b) -> dstT[:, :, b*128:(b+1)*128]."""
        for kc in range(KT):
            tp = zps.tile([128, 128], BF16, tag="z", name="tp")
            nc.tensor.transpose(tp[:], src_sb[:, kc * 128 : (kc + 1) * 128], ident_b[:])
            nc.vector.tensor_copy(dstT[:, kc, b * 128 : (b + 1) * 128], tp[:])

    def accumulate(psums, stat_list, strips, start=True, stop=True, sbuf_mov=None,
                   pre_dma=None, dma_engine=None):
        """Accumulate sum_k of lhsT.T @ rhs into psums[b][j].

        strips: DRAM sources streamed as full-width [128, H] tiles (0.5 MB
        contiguous DMAs keep the HW DGE at large-packet throughput), or
        sbuf_mov: a resident k-major [128, KT, H] SBUF tile used directly.
        Loop order keeps each stationary tile resident for NSL consecutive
        moving slices. pre_dma(kc) lets the caller interleave extra loads.
        dma_engine picks the issuing queue: a dma_start that waits on a
        collective must NOT sit on the Sync queue or it head-of-line blocks
        every later Sync DMA (outputs, W_y stream).
        """
        NP = len(stat_list)
        eng = dma_engine or nc.sync
        for kc in range(KT):
            if pre_dma is not None:
                pre_dma(kc)
            mov = []
            if sbuf_mov is not None:
                mov = [sbuf_mov[:, kc, :]]
            else:
                for dram_src in strips:
                    w = wmov.tile([128, H], BF16, tag="wm", name="wm")
                    eng.dma_start(w[:], dram_src[kc * 128 : (kc + 1) * 128, :])
                    mov.append(w[:])
            for p, statT in enumerate(stat_list):
                for b in range(NB):
                    for j in range(NSL):
                        nc.tensor.matmul(
                            psums[b][j][:],
                            statT[:, kc, b * 128 : (b + 1) * 128],
                            mov[p][:, j * 512 : (j + 1) * 512],
                            start=(start and kc == 0 and p == 0),
                            stop=(stop and kc == KT - 1 and p == NP - 1),
                        )

    # ---- Stage 1: the four gates (i, o first so their AllGathers overlap f,g)
    gate_specs = [
        ("i", AF.Sigmoid),
        ("o", AF.Sigmoid),
        ("f", AF.Sigmoid),
        ("g", AF.Tanh),
    ]
    for gi, (gname, func) in enumerate(gate_specs):
        bx_sb = bxp.tile([128, H], BF16, tag="bx", name="bx")
        nc.sync.dma_start(bx_sb[:], I[f"bx_{gname}"][:])
        ga_sb = be_sb = None
        if apply_affine:
            ga_sb = bxp.tile([128, H], BF16, tag="ga", name="ga")
            nc.sync.dma_start(ga_sb[:], I[f"ga_{gname}"][:])
            be_sb = bxp.tile([128, H], BF16, tag="be", name="be")
            nc.sync.dma_start(be_sb[:], I[f"be_{gname}"][:])

        z_sb = [rows.tile([128, H], F32, tag="z_sb", name="z_sb") for _ in range(NB)]
        psums = [
            [zps.tile([128, 512], F32, tag="z", name="z") for _ in range(NSL)]
            for _ in range(NB)
        ]

        # Interleave the persistent hT/xT chunk loads into the first gate's
        # streaming loop so the PE's first matmul only waits for 4 small DMAs.
        pre = None
        if gi == 0:
            def pre(kc):
                nc.sync.dma_start(hT[:, kc, :], I["hT"][kc * 128 : (kc + 1) * 128, :])
                nc.sync.dma_start(xT[:, kc, :], I["xT"][kc * 128 : (kc + 1) * 128, :])

        accumulate(
            psums,
            [hT, xT],
            [I[f"W_h{gname}"], I[f"W_x{gname}"]],
            pre_dma=pre,
        )
        for b in range(NB):
            for j in range(NSL):
                col = slice(j * 512, (j + 1) * 512)
                nc.vector.tensor_tensor(
                    z_sb[b][:, col], psums[b][j][:], bx_sb[:, col], OP.add
                )
        for b in range(NB):
            gt = rows.tile([128, H], BF16, tag="gate", name="gate")
            layernorm_act(z_sb[b], func, gt, ga_sb, be_sb)
            if gname in ("i", "o"):
                nc.sync.dma_start(io_in[gname][b * 128 : (b + 1) * 128, :], gt[:])
            else:
                transpose_rows(gt, fT if gname == "f" else gT, b)
        if gname in ("i", "o"):
            nc.gpsimd.collective_compute(
                "AllGather",
                OP.bypass,
                replica_groups=[list(range(NCORES))],
                ins=[io_in[gname].opt()],
                outs=[io_full[gname].opt()],
            )

    # ---- Stage 2: c_t = f @ c_states + g @ i ; tanh + transpose
    # Split: all f@c chunks first (no collective dependency), then g@i.
    c_sb = [rows.tile([128, H], BF16, tag="c_sb", name="c_sb") for _ in range(NB)]
    tc_sb = [rows.tile([128, H], BF16, tag="z_sb", name="tc_sb") for _ in range(NB)]
    psums = [
        [zps.tile([128, 512], F32, tag="z", name="z") for _ in range(NSL)]
        for _ in range(NB)
    ]
    accumulate(psums, [fT], [I["c_states"]], start=True, stop=False)
    # i-strips wait on AllGather(i): issue them on the Scalar queue (Scalar's
    # next op, the stage-2 tanh, depends on these matmuls anyway).
    accumulate(psums, [gT], [io_full["i"]], start=False, stop=True,
               dma_engine=nc.scalar)
    # Prefetch the gathered o into SBUF during stage-2 compute. These wait on
    # AllGather(o), so issue them on the otherwise-idle GpSimd queue.
    for kc in range(KT):
        nc.gpsimd.dma_start(oP[:, kc, :], io_full["o"][kc * 128 : (kc + 1) * 128, :])
    for b in range(NB):
        for j in range(NSL):
            col = slice(j * 512, (j + 1) * 512)
            nc.vector.tensor_copy(c_sb[b][:, col], psums[b][j][:])
            nc.scalar.activation(tc_sb[b][:, col], psums[b][j][:], AF.Tanh)
    for b in range(NB):
        nc.sync.dma_start(Outs["c_out"][b * 128 : (b + 1) * 128, :], c_sb[b][:])
        transpose_rows(tc_sb[b], tcT, b)

    # ---- Stage 3: h_t = tanh(c_t) @ o ; transpose (o is SBUF-resident)
    h_sb = [rows.tile([128, H], BF16, tag="gate", name="h_sb") for _ in range(NB)]
    psums = [
        [zps.tile([128, 512], F32, tag="z", name="z") for _ in range(NSL)]
        for _ in range(NB)
    ]
    accumulate(psums, [tcT], None, sbuf_mov=oP)
    for b in range(NB):
        for j in range(NSL):
            col = slice(j * 512, (j + 1) * 512)
            nc.vector.tensor_copy(h_sb[b][:, col], psums[b][j][:])
    for b in range(NB):
        nc.sync.dma_start(Outs["h_out"][b * 128 : (b + 1) * 128, :], h_sb[b][:])
        transpose_rows(h_sb[b], htT, b)

    # ---- Stage 4: y = h_t @ W_y + b_y  (W_y streams during stage 3)
    by_sb = bxp.tile([128, OD], BF16, tag="bx", name="bx")
    nc.sync.dma_start(by_sb[:], I["by_rep"][:])
    y_sb = [rows.tile([128, OD], BF16, tag="z_sb", name="y_sb") for _ in range(NB)]
    psums = [
        [zps.tile([128, 512], F32, tag="z", name="z") for _ in range(NSL)]
        for _ in range(NB)
    ]
    accumulate(psums, [htT], [I["W_y"]])
    for b in range(NB):
        for j in range(NSL):
            col = slice(j * 512, (j + 1) * 512)
            nc.vector.tensor_tensor(
                y_sb[b][:, col], psums[b][j][:], by_sb[:, col], OP.add
            )
    for b in range(NB):
        nc.sync.dma_start(Outs["y_out"][b * 128 : (b + 1) * 128, :], y_sb[b][:])


def _build(apply_affine: bool):
    nc = bacc.Bacc(
        "TRN2",
        target_bir_lowering=False,
        debug=False,
        enable_asserts=False,
        num_devices=NCORES,
    )
    I = {}

    def di(name, shape, dt=BF16):
        I[name] = nc.dram_tensor(name, list(shape), dt, kind="ExternalInput").ap()

    di("hT", (H, BS))
    di("xT", (H, BS))
    di("c_states", (B, H))
    di("W_y", (H, OD))
    di("by_rep", (128, OD))
    for g in "fgio":
        di(f"W_h{g}", (H, H))
        di(f"W_x{g}", (H, H))
        di(f"bx_{g}", (128, H))
        if apply_affine:
            di(f"ga_{g}", (128, H))
            di(f"be_{g}", (128, H))
    Outs = {
        n: nc.dram_tensor(n, [BS, H], BF16, kind="ExternalOutput").ap()
        for n in ("c_out", "h_out", "y_out")
    }

    with tile.TileContext(nc) as tc, ExitStack() as ctx:
        _body(ctx, tc, I, Outs, apply_affine)
    nc.compile()
    return nc


def kernel(**inputs):
    inputs = {k: np.asarray(v, dtype=np.float32) for k, v in inputs.items()}
    apply_affine = not all(
        np.all(inputs[f"g_{g}"] == 1.0) and np.all(inputs[f"be_{g}"] == 0.0)
        for g in "fgio"
    )
    if apply_affine not in _cache:
        _cache[apply_affine] = _build(apply_affine)
    nc = _cache[apply_affine]

    import ml_dtypes

    bf16 = ml_dtypes.bfloat16
    hT_full = np.ascontiguousarray(inputs["h_states"].T.astype(bf16))
    xT_full = np.ascontiguousarray(inputs["inputs"].T.astype(bf16))

    def rep(v):
        return np.ascontiguousarray(
            np.broadcast_to(v[None, :].astype(bf16), (128, v.shape[0]))
        )

    base = {
        "c_states": inputs["c_states"].astype(bf16),
        "W_y": inputs["W_y"].astype(bf16),
        "by_rep": rep(inputs["b_y"]),
    }
    for g in "fgio":
        base[f"W_h{g}"] = inputs[f"W_h{g}"].astype(bf16)
        base[f"W_x{g}"] = inputs[f"W_x{g}"].astype(bf16)
        base[f"bx_{g}"] = rep(inputs[f"b_x{g}"])
        if apply_affine:
            base[f"ga_{g}"] = rep(inputs[f"g_{g}"])
            base[f"be_{g}"] = rep(inputs[f"be_{g}"])

    in_maps = [
        dict(
            base,
            hT=np.ascontiguousarray(hT_full[:, c * BS : (c + 1) * BS]),
            xT=np.ascontiguousarray(xT_full[:, c * BS : (c + 1) * BS]),
        )
        for c in range(NCORES)
    ]

    res = run_bass_kernel_spmd(
        nc,
        in_maps,
        list(range(NCORES)),
        trace=bool(os.environ.get("KERNEL_TRACE")),
    )
    kernel.last_result = res

    def cat(name):
        return np.concatenate(
            [res.results[c][name].astype(np.float32) for c in range(NCORES)], axis=0
        )

    return (cat("c_out"), cat("h_out"), cat("y_out"))


# revision 13
# speedup vs baseline: 1.3400x; 1.0030x over previous
"""LSTMCell-variant Bass kernel for 8 Trainium2 NeuronCores.

Reference computation (B = H = O = 2048, fp32):
    z_g  = h @ W_hg + x @ W_xg + b_xg          (4 gates g in {f,g,i,o})
    gate = act(LayerNorm(z_g))                  (sigmoid/tanh/sigmoid/sigmoid)
    c_t  = f @ c_states + g @ i                 (matmul gating, not elementwise)
    h_t  = tanh(c_t) @ o
    y_t  = h_t @ W_y + b_y
    returns (c_t, h_t, y_t)

Strategy: data-parallel over batch rows (256 rows/core). Each core computes
its 4 gate row-shards; i and o are AllGathered (they are the RIGHT operands
of the gate matmuls, so every core needs them in full); f, g, tanh(c_t), h_t
are only needed as row shards (transposed locally on the PE for use as the
stationary matmul operand).

Precision plan: everything is bf16 (fp32 PSUM accumulate) except stage 3
(h_t = tanh(c_t) @ o), which runs fp8(e4m3) with DoubleRow (2 k-rows per
PE pass): its operands live in (-1,1) and h_t's absmax-rel budget has ~2x
slack. Stage 4 must stay bf16: h_t's sigma is ~122, so fp8 quantization of
either h_t or W_y alone already exceeds y_t's error budget (measured).

Scheduling keeps the PE dense (any >3.4us idle window halves the PE clock
via the HAM activity monitor) and keeps the collectives fed:
  - gate order i, o, f, g so both AllGathers overlap the f/g projections;
  - stage 2 runs all f@c chunks before g@i so the i-gather has extra slack;
  - any dma_start that waits on a collective is issued on a non-Sync queue
    (Scalar/GpSimd) to avoid head-of-line blocking the output/weight DMAs;
  - o (fp8) and W_y (bf16) are prefetched into SBUF during stage 2, so
    stages 3-4 issue no input DMA at all;
  - LayerNorm uses the E[z^2]-m^2 form: both reductions read the raw z in
    parallel (Vector + Scalar) and the normalize+activate folds into one
    Scalar activation via its per-partition scale/bias operands.

Host-side staging (free w.r.t. HW time): x and h are fed pre-transposed
(contraction dim on partitions); biases pre-replicated to 128 partitions;
all dtype casts and the W_y/b_y/y scalings happen on the host.
"""

import os
from contextlib import ExitStack

import numpy as np

os.environ.setdefault("MYCRO_LOCAL_CACHE", "1")

try:
    import concourse.bass as bass  # noqa: F401
except ImportError:  # pragma: no cover
    import sys

    sys.path.insert(0, "/opt/trn_rl_repo")
    import concourse.bass as bass  # noqa: F401

import concourse.mybir as mybir
import concourse.tile as tile
from concourse import bacc
from concourse.bass_utils import run_bass_kernel_spmd
from concourse.masks import make_identity

B = 2048
H = 2048
OD = 2048
NCORES = 8
BS = B // NCORES  # 256 batch rows per core
NB = BS // 128  # 2 row-chunks of 128
KT = H // 128  # 16 contraction chunks
NSL = 4  # moving slices of 512 per full-width strip
EPS = 1e-5

F32 = mybir.dt.float32
BF16 = mybir.dt.bfloat16
F8 = mybir.dt.float8e4
AX = mybir.AxisListType
OP = mybir.AluOpType
AF = mybir.ActivationFunctionType
DR = mybir.MatmulPerfMode.DoubleRow

_cache = {}


def _body(ctx: ExitStack, tc, I, Outs, apply_affine: bool):
    nc = tc.nc

    const = ctx.enter_context(tc.tile_pool(name="const", bufs=1))
    persist = ctx.enter_context(tc.tile_pool(name="persist", bufs=1))
    wmov = ctx.enter_context(tc.tile_pool(name="wmov", bufs=6))
    bxp = ctx.enter_context(tc.tile_pool(name="bxp", bufs=2))
    rows = ctx.enter_context(tc.tile_pool(name="rows", bufs=2))
    stats = ctx.enter_context(tc.tile_pool(name="stats", bufs=6))
    zps = ctx.enter_context(tc.tile_pool(name="zps", bufs=8, space="PSUM"))
    dram = ctx.enter_context(tc.tile_pool(name="dram", bufs=1, space="DRAM"))

    ident = const.tile([128, 128], F32, tag="ident", name="ident")
    make_identity(nc, ident[:])
    ident_b = const.tile([128, 128], BF16, tag="ident_b", name="ident_b")
    nc.vector.tensor_copy(ident_b[:], ident[:])
    epsb = const.tile([128, 1], F32, tag="epsb", name="epsb")
    nc.gpsimd.memset(epsb[:], EPS)

    # Persistent k-major activations: [128 partitions, KT chunks, col block].
    # Column block kc of hT holds hT[kc*128:(kc+1)*128, :] i.e. contraction
    # rows on partitions, ready to slice as a [128, 128] stationary operand.
    def kmajor(name, tag=None, cols=BS, dt=BF16):
        return persist.tile([128, KT, cols], dt, tag=tag or name, name=name)

    hT = kmajor("hT")
    xT = kmajor("xT")
    fT = kmajor("fT")
    gT = kmajor("gT")
    # hT/xT are dead once the gate projections finish; tcT/htT are only
    # written afterwards, so they share the same SBUF slots.
    tcT = kmajor("tcT", tag="hT", dt=F8)
    htT = kmajor("htT", tag="xT")
    # fp8 o gate (gathered) and bf16 W_y, both SBUF-resident before stage 3
    # so stages 3-4 issue no input DMA.
    oP = kmajor("oP", cols=H, dt=F8)
    wyP = kmajor("wyP", cols=OD)

    # DRAM bounce buffers for the i/o AllGathers.
    io_dt = {"i": BF16, "o": F8}
    io_in = {g: dram.tile([BS, H], io_dt[g], tag=f"io_in_{g}", name=f"io_in_{g}") for g in "io"}
    io_full = {g: dram.tile([B, H], io_dt[g], tag=f"io_full_{g}", name=f"io_full_{g}", addr_space="Shared") for g in "io"}

    def layernorm_act(z_sb, func, gate_out, ga_sb, be_sb):
        """z_sb [128, H] -> gate_out [128, H] = func(LN(z)) (affine optional).

        Uses var = E[z^2] - mean^2 so the two reductions over z run
        concurrently on Vector and Scalar, then normalize+activate is a
        single Scalar pass: func(z*inv + (-m*inv)).
        """
        s1 = stats.tile([128, 1], F32, tag="s1", name="s1")
        nc.vector.tensor_reduce(s1[:], z_sb[:], AX.X, OP.add)
        trash = rows.tile([128, H], BF16, tag="gate", name="trash")
        ssq = stats.tile([128, 1], F32, tag="ssq", name="ssq")
        nc.scalar.activation(trash[:], z_sb[:], AF.Square, accum_out=ssq[:])
        m = stats.tile([128, 1], F32, tag="m", name="m")
        nc.vector.tensor_scalar_mul(m[:], s1[:], 1.0 / H)
        v = stats.tile([128, 1], F32, tag="v", name="v")
        nc.vector.tensor_scalar_mul(v[:], ssq[:], 1.0 / H)
        mm = stats.tile([128, 1], F32, tag="mm", name="mm")
        nc.vector.tensor_tensor(mm[:], m[:], m[:], OP.mult)
        nc.vector.tensor_tensor(v[:], v[:], mm[:], OP.subtract)
        std = stats.tile([128, 1], F32, tag="std", name="std")
        nc.scalar.activation(std[:], v[:], AF.Sqrt, bias=epsb[:])
        inv = stats.tile([128, 1], F32, tag="inv", name="inv")
        nc.vector.reciprocal(inv[:], std[:])
        nmi = stats.tile([128, 1], F32, tag="nmi", name="nmi")
        nc.vector.tensor_tensor(nmi[:], m[:], inv[:], OP.mult)
        nc.vector.tensor_scalar_mul(nmi[:], nmi[:], -1.0)
        if not apply_affine:
            nc.scalar.activation(gate_out[:], z_sb[:], func, scale=inv[:], bias=nmi[:])
        else:
            nc.vector.tensor_scalar(
                out=z_sb[:], in0=z_sb[:], scalar1=inv[:], scalar2=nmi[:],
                op0=OP.mult, op1=OP.add,
            )
            nc.vector.tensor_tensor(z_sb[:], z_sb[:], ga_sb[:], OP.mult)
            nc.vector.tensor_tensor(z_sb[:], z_sb[:], be_sb[:], OP.add)
            nc.scalar.activation(gate_out[:], z_sb[:], func)

    def transpose_rows(src_sb, dstT, b):
        """src_sb [128, H] bf16 (row-chunk b) -> dstT[:, :, b*128:(b+1)*128].

        The PE transpose runs in bf16 (fp8 transpose mode is rejected by the
        compiler); the psum->SBUF copy casts to dstT's dtype (bf16 or fp8).
        """
        for kc in range(KT):
            tp = zps.tile([128, 128], BF16, tag="z", name="tp")
            nc.tensor.transpose(tp[:], src_sb[:, kc * 128 : (kc + 1) * 128], ident_b[:])
            nc.vector.tensor_copy(dstT[:, kc, b * 128 : (b + 1) * 128], tp[:])

    def accumulate(psums, stat_list, strips, start=True, stop=True,
                   pre_dma=None, dma_engine=None, sbuf_mov=None):
        """Accumulate sum_k of lhsT.T @ rhs into psums[b][j].

        strips: DRAM sources streamed as full-width [128, H] tiles (0.5 MB
        contiguous DMAs keep the HW DGE at large-packet throughput).
        Loop order keeps each stationary tile resident for NSL consecutive
        moving slices. pre_dma(kc) lets the caller interleave extra loads.
        dma_engine picks the issuing queue: a dma_start that waits on a
        collective must NOT sit on the Sync queue or it head-of-line blocks
        every later Sync DMA (outputs, W_y stream).
        """
        NP = len(stat_list)
        eng = dma_engine or nc.sync
        for kc in range(KT):
            if pre_dma is not None:
                pre_dma(kc)
            mov = []
            if sbuf_mov is not None:
                mov = [sbuf_mov[:, kc, :]]
            strips = [] if sbuf_mov is not None else strips
            for dram_src in strips:
                w = wmov.tile([128, H], BF16, tag="wm", name="wm")
                eng.dma_start(w[:], dram_src[kc * 128 : (kc + 1) * 128, :])
                mov.append(w[:])
            for p, statT in enumerate(stat_list):
                for b in range(NB):
                    for j in range(NSL):
                        nc.tensor.matmul(
                            psums[b][j][:],
                            statT[:, kc, b * 128 : (b + 1) * 128],
                            mov[p][:, j * 512 : (j + 1) * 512],
                            start=(start and kc == 0 and p == 0),
                            stop=(stop and kc == KT - 1 and p == NP - 1),
                        )

    def accumulate_dr(psums, statT, movP):
        """fp8 DoubleRow accumulate: statT/movP are SBUF-resident k-major fp8
        tiles; adjacent k-chunk pairs ride one PE pass (2 rows per cell)."""
        for kci in range(0, KT, 2):
            for b in range(NB):
                for j in range(NSL):
                    nc.tensor.matmul(
                        psums[b][j][:],
                        statT[:, kci : kci + 2, b * 128 : (b + 1) * 128],
                        movP[:, kci : kci + 2, j * 512 : (j + 1) * 512],
                        start=(kci == 0),
                        stop=(kci == KT - 2),
                        perf_mode=DR,
                    )

    # ---- Stage 1: the four gates (i, o first so their AllGathers overlap f,g)
    gate_specs = [
        ("i", AF.Sigmoid),
        ("o", AF.Sigmoid),
        ("f", AF.Sigmoid),
        ("g", AF.Tanh),
    ]
    for gi, (gname, func) in enumerate(gate_specs):
        bx_sb = bxp.tile([128, H], BF16, tag="bx", name="bx")
        nc.sync.dma_start(bx_sb[:], I[f"bx_{gname}"][:])
        ga_sb = be_sb = None
        if apply_affine:
            ga_sb = bxp.tile([128, H], BF16, tag="ga", name="ga")
            nc.sync.dma_start(ga_sb[:], I[f"ga_{gname}"][:])
            be_sb = bxp.tile([128, H], BF16, tag="be", name="be")
            nc.sync.dma_start(be_sb[:], I[f"be_{gname}"][:])

        z_sb = [rows.tile([128, H], F32, tag="z_sb", name="z_sb") for _ in range(NB)]
        psums = [
            [zps.tile([128, 512], F32, tag="z", name="z") for _ in range(NSL)]
            for _ in range(NB)
        ]

        # Interleave the persistent hT/xT chunk loads into the first gate's
        # streaming loop so the PE's first matmul only waits for 4 small DMAs.
        pre = None
        if gi == 0:
            def pre(kc):
                nc.sync.dma_start(hT[:, kc, :], I["hT"][kc * 128 : (kc + 1) * 128, :])
                nc.sync.dma_start(xT[:, kc, :], I["xT"][kc * 128 : (kc + 1) * 128, :])

        accumulate(
            psums,
            [hT, xT],
            [I[f"W_h{gname}"], I[f"W_x{gname}"]],
            pre_dma=pre,
        )
        for b in range(NB):
            for j in range(NSL):
                col = slice(j * 512, (j + 1) * 512)
                nc.vector.tensor_tensor(
                    z_sb[b][:, col], psums[b][j][:], bx_sb[:, col], OP.add
                )
        for b in range(NB):
            gt = rows.tile([128, H], io_dt.get(gname, BF16), tag="gate", name="gate")
            layernorm_act(z_sb[b], func, gt, ga_sb, be_sb)
            if gname in ("i", "o"):
                nc.sync.dma_start(io_in[gname][b * 128 : (b + 1) * 128, :], gt[:])
            else:
                transpose_rows(gt, fT if gname == "f" else gT, b)
        if gname in ("i", "o"):
            nc.gpsimd.collective_compute(
                "AllGather",
                OP.bypass,
                replica_groups=[list(range(NCORES))],
                ins=[io_in[gname].opt()],
                outs=[io_full[gname].opt()],
            )

    # ---- Stage 2: c_t = f @ c_states + g @ i ; tanh + transpose
    # Split: all f@c chunks first (no collective dependency), then g@i.
    c_sb = [rows.tile([128, H], BF16, tag="c_sb", name="c_sb") for _ in range(NB)]
    tc_sb = [rows.tile([128, H], BF16, tag="z_sb", name="tc_sb") for _ in range(NB)]
    psums = [
        [zps.tile([128, 512], F32, tag="z", name="z") for _ in range(NSL)]
        for _ in range(NB)
    ]
    accumulate(psums, [fT], [I["c_states"]], start=True, stop=False)
    # i-strips wait on AllGather(i): issue them on the Scalar queue (Scalar's
    # next op, the stage-2 tanh, depends on these matmuls anyway).
    accumulate(psums, [gT], [io_full["i"]], start=False, stop=True,
               dma_engine=nc.scalar)
    # Prefetch the gathered fp8 o into SBUF during stage-2 compute. These wait
    # on AllGather(o), so issue them on the otherwise-idle GpSimd queue.
    for kc in range(KT):
        nc.gpsimd.dma_start(oP[:, kc, :], io_full["o"][kc * 128 : (kc + 1) * 128, :])
    # W_y (bf16) has no dependencies; on the Scalar queue after the i-strips
    # it issues as soon as AllGather(i) lands, giving it the whole of stage
    # 2-3 to stream in before stage 4.
    for kc in range(KT):
        nc.scalar.dma_start(wyP[:, kc, :], I["W_y"][kc * 128 : (kc + 1) * 128, :])
    for b in range(NB):
        for j in range(NSL):
            col = slice(j * 512, (j + 1) * 512)
            nc.vector.tensor_copy(c_sb[b][:, col], psums[b][j][:])
            nc.scalar.activation(tc_sb[b][:, col], psums[b][j][:], AF.Tanh)
    for b in range(NB):
        nc.sync.dma_start(Outs["c_out"][b * 128 : (b + 1) * 128, :], c_sb[b][:])
        transpose_rows(tc_sb[b], tcT, b)

    # ---- Stage 3: h_t = tanh(c_t) @ o  (fp8 DoubleRow, all SBUF-resident)
    h_sb = [rows.tile([128, H], BF16, tag="gate", name="h_sb") for _ in range(NB)]
    psums = [
        [zps.tile([128, 512], F32, tag="z", name="z") for _ in range(NSL)]
        for _ in range(NB)
    ]
    accumulate_dr(psums, tcT, oP)
    for b in range(NB):
        for j in range(NSL):
            col = slice(j * 512, (j + 1) * 512)
            nc.vector.tensor_copy(h_sb[b][:, col], psums[b][j][:])
    for b in range(NB):
        nc.sync.dma_start(Outs["h_out"][b * 128 : (b + 1) * 128, :], h_sb[b][:])
        transpose_rows(h_sb[b], htT, b)

    # ---- Stage 4: y = h_t @ W_y + b_y  (bf16; W_y is SBUF-resident)
    by_sb = bxp.tile([128, OD], BF16, tag="bx", name="bx")
    nc.sync.dma_start(by_sb[:], I["by_rep"][:])
    y_sb = [rows.tile([128, OD], BF16, tag="z_sb", name="y_sb") for _ in range(NB)]
    psums = [
        [zps.tile([128, 512], F32, tag="z", name="z") for _ in range(NSL)]
        for _ in range(NB)
    ]
    accumulate(psums, [htT], None, sbuf_mov=wyP)
    for b in range(NB):
        for j in range(NSL):
            col = slice(j * 512, (j + 1) * 512)
            nc.vector.tensor_tensor(
                y_sb[b][:, col], psums[b][j][:], by_sb[:, col], OP.add
            )
    for b in range(NB):
        nc.sync.dma_start(Outs["y_out"][b * 128 : (b + 1) * 128, :], y_sb[b][:])


def _build(apply_affine: bool):
    nc = bacc.Bacc(
        "TRN2",
        target_bir_lowering=False,
        debug=False,
        enable_asserts=False,
        num_devices=NCORES,
    )
    I = {}

    def di(name, shape, dt=BF16):
        I[name] = nc.dram_tensor(name, list(shape), dt, kind="ExternalInput").ap()

    di("hT", (H, BS))
    di("xT", (H, BS))
    di("c_states", (B, H))
    di("W_y", (H, OD))
    di("by_rep", (128, OD))
    for g in "fgio":
        di(f"W_h{g}", (H, H))
        di(f"W_x{g}", (H, H))
        di(f"bx_{g}", (128, H))
        if apply_affine:
            di(f"ga_{g}", (128, H))
            di(f"be_{g}", (128, H))
    Outs = {
        n: nc.dram_tensor(n, [BS, H], BF16, kind="ExternalOutput").ap()
        for n in ("c_out", "h_out", "y_out")
    }

    with tile.TileContext(nc) as tc, ExitStack() as ctx:
        _body(ctx, tc, I, Outs, apply_affine)
    nc.compile()
    return nc


def kernel(**inputs):
    inputs = {k: np.asarray(v, dtype=np.float32) for k, v in inputs.items()}
    apply_affine = not all(
        np.all(inputs[f"g_{g}"] == 1.0) and np.all(inputs[f"be_{g}"] == 0.0)
        for g in "fgio"
    )
    if apply_affine not in _cache:
        _cache[apply_affine] = _build(apply_affine)
    nc = _cache[apply_affine]

    import ml_dtypes

    bf16 = ml_dtypes.bfloat16
    f8 = ml_dtypes.float8_e4m3
    hT_full = np.ascontiguousarray(inputs["h_states"].T.astype(bf16))
    xT_full = np.ascontiguousarray(inputs["inputs"].T.astype(bf16))

    def rep(v, dt=bf16):
        return np.ascontiguousarray(
            np.broadcast_to(v[None, :].astype(dt), (128, v.shape[0]))
        )

    base = {
        "c_states": inputs["c_states"].astype(bf16),
        "W_y": inputs["W_y"].astype(bf16),
        "by_rep": rep(inputs["b_y"]),
    }
    for g in "fgio":
        base[f"W_h{g}"] = inputs[f"W_h{g}"].astype(bf16)
        base[f"W_x{g}"] = inputs[f"W_x{g}"].astype(bf16)
        base[f"bx_{g}"] = rep(inputs[f"b_x{g}"])
        if apply_affine:
            base[f"ga_{g}"] = rep(inputs[f"g_{g}"])
            base[f"be_{g}"] = rep(inputs[f"be_{g}"])

    in_maps = [
        dict(
            base,
            hT=np.ascontiguousarray(hT_full[:, c * BS : (c + 1) * BS]),
            xT=np.ascontiguousarray(xT_full[:, c * BS : (c + 1) * BS]),
        )
        for c in range(NCORES)
    ]

    res = run_bass_kernel_spmd(
        nc,
        in_maps,
        list(range(NCORES)),
        trace=bool(os.environ.get("KERNEL_TRACE")),
    )
    kernel.last_result = res

    def cat(name, scale=1.0):
        return np.concatenate(
            [
                res.results[c][name].astype(np.float32) * scale
                for c in range(NCORES)
            ],
            axis=0,
        )

    return (cat("c_out"), cat("h_out"), cat("y_out"))
